# revision 5
# baseline (speedup 1.0000x reference)
"""KV cache paged scatter update (nn_KVCacheManager) on 8 TRN2 NeuronCores.

Reference semantics (B=32, T=512, H=8, D=128, PAGE_SIZE=256):
  for each (b, i): abs = cache_seqlens[b] + i
    page = page_table[b, abs // 256]; slot = abs % 256
    k_cache[page, slot] = k[b, i]; v_cache[page, slot] = v[b, i]
  new_seqlens = cache_seqlens + T

Sharding: data parallel over batch — core c owns sequences 4c..4c+3 and
their pages, scattering locally with zero communication.

Device kernel (one SPMD program, offsets are runtime DATA, not constants):
  Each sequence writes T=512 consecutive tokens starting at abs offset
  cache_seqlens[b]. When a sequence's pages are consecutive in the page
  table (they are: page_table is arange), its destination is one
  contiguous 512-row range inside its private 2048-row region. So the
  scatter is one dynamic-offset 4 MiB copy per sequence (k and v stacked
  on the AP's outer dim), with the row offset loaded from an int32 input
  tensor into an SP register at runtime and hardware-bounds-checked.
Host does only what a production paged-KV host does: computes the int32
slot mapping (a few hundred ints) and assembles the final numpy output.
"""

import numpy as np

import concourse.bass as bass
import concourse.mybir as mybir
from concourse.bass_utils import run_bass_kernel_spmd

B, T, H, D = 32, 512, 8, 128
PAGE_SIZE = 256
PAGES_PER_SEQ = 8
NUM_PAGES = B * PAGES_PER_SEQ  # 256
N_CORES = 8
SPC = B // N_CORES             # 4 sequences per core
REGION = PAGES_PER_SEQ * PAGE_SIZE  # 2048 rows (slots) per sequence
ROW = H * D                    # 1024 f32 per slot (4 KiB)

_PROGRAM = None  # compiled Bass program cache (module-level, one build)


def _build_program():
    """One SPMD program for all 8 cores. Per-core behavior differs only in
    input data: the kv shard and the dst_row offsets.

    kv_src[0] = k tokens, kv_src[1] = v tokens (SPC*T rows of 4 KiB each).
    kv_out[0] = k cache regions, kv_out[1] = v cache regions
    (SPC sequences x 2048 rows). ExternalOutput buffers are zero-filled by
    the runtime; only the written rows are consumed by the host. One DMA
    per sequence moves both k and v (outer AP dim) to the dynamic row
    offset. Few DMA instructions matters: the TileContext tail drain
    encodes one sync-wait per DMA sem lane and overflows past ~8.
    """
    nc = bass.Bass()
    f32 = mybir.dt.float32
    i32 = mybir.dt.int32

    kv_src = nc.dram_tensor("kv_src", (2, SPC * T, ROW), f32, kind="ExternalInput")
    # dst_row[0, b] = b*REGION + cache_seqlens[global b]
    dst_row = nc.dram_tensor("dst_row", (1, SPC), i32, kind="ExternalInput")
    kv_out = nc.dram_tensor(
        "kv_out", (2, SPC * REGION, ROW), f32, kind="ExternalOutput"
    )

    # Raw Bass (no TileContext): the Tile tail drain encodes one sync-wait
    # per outstanding sem and overflows the TPB_CTRL wait-slot limit with
    # more than one dynamic DMA. Here everything runs on the SP sequencer
    # in program order with one shared DMA semaphore.
    with (
        nc.sbuf_tensor([1, SPC], i32) as off_tile,
        nc.semaphore() as dma_sem,
        nc.sync.register("off0") as r0,
        nc.sync.register("off1") as r1,
        nc.sync.register("off2") as r2,
        nc.sync.register("off3") as r3,
        nc.Block() as block,
    ):
        regs = [r0, r1, r2, r3]

        @block.sync
        def _(sync):
            sync.dma_start(off_tile[:, :], dst_row[:, :]).then_inc(dma_sem, 16)
            sync.wait_ge(dma_sem, 16)
            for b in range(SPC):
                sync.reg_load(regs[b], off_tile[0:1, b : b + 1])
                rv = sync.snap(regs[b])
                # Dynamic DRAM dst offset: dma_start auto-enables the
                # hardware bounds check (bounds_check="err").
                sync.dma_start(
                    kv_out[:, bass.ds(rv, T), :],
                    kv_src[:, b * T : (b + 1) * T, :],
                ).then_inc(dma_sem, 16)
            sync.wait_ge(dma_sem, 16 * (SPC + 1))

    return nc


def _get_program():
    global _PROGRAM
    if _PROGRAM is None:
        _PROGRAM = _build_program()
    return _PROGRAM


def _seq_page_base(page_table):
    """If every sequence's pages are consecutive (page_table[b, j] ==
    page_table[b, 0] + j), return the (B,) int array of base pages, else
    None (→ general numpy fallback)."""
    base = page_table[:, :1]
    expect = base + np.arange(PAGES_PER_SEQ, dtype=page_table.dtype)[None, :]
    if np.array_equal(page_table, expect):
        return page_table[:, 0].astype(np.int64)
    return None


def _numpy_fallback(k, v, k_cache, v_cache, page_table, cache_seqlens):
    abs_pos = cache_seqlens[:, None].astype(np.int64) + np.arange(T)[None, :]
    page_idx = np.take_along_axis(
        page_table.astype(np.int64), abs_pos // PAGE_SIZE, axis=1
    )
    slot = abs_pos % PAGE_SIZE
    new_k = np.array(k_cache, copy=True)
    new_v = np.array(v_cache, copy=True)
    new_k[page_idx, slot] = k
    new_v[page_idx, slot] = v
    return new_k, new_v, (cache_seqlens + np.int32(T)).astype(np.int32)


def kernel(k, v, k_cache, v_cache, page_table, cache_seqlens):
    k = np.ascontiguousarray(k, dtype=np.float32)
    v = np.ascontiguousarray(v, dtype=np.float32)
    page_table = np.asarray(page_table)
    cache_seqlens = np.asarray(cache_seqlens)

    base_pages = _seq_page_base(page_table)
    if base_pages is None:
        return _numpy_fallback(k, v, k_cache, v_cache, page_table, cache_seqlens)

    nc = _get_program()

    seql = cache_seqlens.astype(np.int64)
    kf = k.reshape(B, T * ROW)
    vf = v.reshape(B, T * ROW)
    in_maps = []
    for c in range(N_CORES):
        gb = slice(c * SPC, (c + 1) * SPC)
        dst = (np.arange(SPC, dtype=np.int64) * REGION + seql[gb]).astype(np.int32)
        kv = np.stack([kf[gb].reshape(SPC * T, ROW), vf[gb].reshape(SPC * T, ROW)])
        in_maps.append({"kv_src": kv, "dst_row": dst.reshape(1, SPC)})

    res = run_bass_kernel_spmd(nc, in_maps, core_ids=list(range(N_CORES)))

    new_k = np.array(k_cache, copy=True).reshape(NUM_PAGES * PAGE_SIZE, ROW)
    new_v = np.array(v_cache, copy=True).reshape(NUM_PAGES * PAGE_SIZE, ROW)
    for c in range(N_CORES):
        kv_out = res.results[c]["kv_out"]
        for bl in range(SPC):
            g = c * SPC + bl
            s = int(seql[g])
            dst0 = int(base_pages[g]) * PAGE_SIZE + s
            src0 = bl * REGION + s
            new_k[dst0 : dst0 + T] = kv_out[0, src0 : src0 + T]
            new_v[dst0 : dst0 + T] = kv_out[1, src0 : src0 + T]

    new_k = new_k.reshape(NUM_PAGES, PAGE_SIZE, H, D)
    new_v = new_v.reshape(NUM_PAGES, PAGE_SIZE, H, D)
    new_seqlens = (cache_seqlens + np.int32(T)).astype(np.int32)
    return new_k, new_v, new_seqlens


# revision 8
# speedup vs baseline: 278.6482x; 278.6482x over previous
"""KV cache paged scatter update (nn_KVCacheManager) on 8 TRN2 NeuronCores.

Reference semantics (B=32, T=512, H=8, D=128, PAGE_SIZE=256):
  for each (b, i): abs = cache_seqlens[b] + i
    page = page_table[b, abs // 256]; slot = abs % 256
    k_cache[page, slot] = k[b, i]; v_cache[page, slot] = v[b, i]
  new_seqlens = cache_seqlens + T

Sharding: data parallel over batch — core c owns sequences 4c..4c+3 and
their pages, scattering locally with zero communication.

Device kernel (one SPMD program, offsets are runtime DATA, not constants):
  Each sequence writes T=512 consecutive tokens starting at abs offset
  cache_seqlens[b]. When a sequence's pages are consecutive in the page
  table (they are: page_table is arange), its destination is one
  contiguous 512-row range inside its private 2048-row region. So the
  scatter is one dynamic-offset 4 MiB copy per sequence (k and v stacked
  on the AP's outer dim), with the row offset loaded from an int32 input
  tensor into an SP register at runtime and hardware-bounds-checked.
Host does only what a production paged-KV host does: computes the int32
slot mapping (a few hundred ints) and assembles the final numpy output.
"""

import numpy as np

import concourse.bass as bass
import concourse.mybir as mybir
from concourse.bass_utils import run_bass_kernel_spmd

B, T, H, D = 32, 512, 8, 128
PAGE_SIZE = 256
PAGES_PER_SEQ = 8
NUM_PAGES = B * PAGES_PER_SEQ  # 256
N_CORES = 8
SPC = B // N_CORES             # 4 sequences per core
REGION = PAGES_PER_SEQ * PAGE_SIZE  # 2048 rows (slots) per sequence
ROW = H * D                    # 1024 f32 per slot (4 KiB)

_PROGRAM = None  # compiled Bass program cache (module-level, one build)


def _build_program():
    """One SPMD program for all 8 cores. Per-core behavior differs only in
    input data: the kv shard and the dst_row offsets.

    kv_src[0] = k tokens, kv_src[1] = v tokens (SPC*T rows of 4 KiB each).
    kv_out[0] = k cache regions, kv_out[1] = v cache regions
    (SPC sequences x 2048 rows). ExternalOutput buffers are zero-filled by
    the runtime; only the written rows are consumed by the host. One DMA
    per sequence moves both k and v (outer AP dim) to the dynamic row
    offset. Few DMA instructions matters: the TileContext tail drain
    encodes one sync-wait per DMA sem lane and overflows past ~8.
    """
    nc = bass.Bass()
    f32 = mybir.dt.float32
    i32 = mybir.dt.int32

    kv_src = nc.dram_tensor("kv_src", (2, SPC * T, ROW), f32, kind="ExternalInput")
    # dst_row[0, b] = b*REGION + cache_seqlens[global b]
    dst_row = nc.dram_tensor("dst_row", (1, SPC), i32, kind="ExternalInput")
    kv_out = nc.dram_tensor(
        "kv_out", (2, SPC * REGION, ROW), f32, kind="ExternalOutput"
    )

    # Raw Bass (no TileContext): the Tile tail drain encodes one sync-wait
    # per outstanding sem and overflows the TPB_CTRL wait-slot limit with
    # more than one dynamic DMA. Here everything runs on the SP sequencer
    # in program order with one shared DMA semaphore.
    with (
        nc.sbuf_tensor([1, SPC], i32) as off_tile,
        nc.semaphore() as dma_sem,
        nc.sync.register("off0") as r0,
        nc.sync.register("off1") as r1,
        nc.sync.register("off2") as r2,
        nc.sync.register("off3") as r3,
        nc.Block() as block,
    ):
        regs = [r0, r1, r2, r3]

        @block.sync
        def _(sync):
            sync.dma_start(off_tile[:, :], dst_row[:, :]).then_inc(dma_sem, 16)
            sync.wait_ge(dma_sem, 16)
            for b in range(SPC):
                sync.reg_load(regs[b], off_tile[0:1, b : b + 1])
                rv = sync.snap(regs[b])
                # Dynamic DRAM dst offset: dma_start auto-enables the
                # hardware bounds check (bounds_check="err").
                sync.dma_start(
                    kv_out[:, bass.ds(rv, T), :],
                    kv_src[:, b * T : (b + 1) * T, :],
                ).then_inc(dma_sem, 16)
            sync.wait_ge(dma_sem, 16 * (SPC + 1))

    return nc


def _get_program():
    global _PROGRAM
    if _PROGRAM is None:
        _PROGRAM = _build_program()
    return _PROGRAM


def _seq_page_base(page_table):
    """If every sequence's pages are consecutive (page_table[b, j] ==
    page_table[b, 0] + j), return the (B,) int array of base pages, else
    None (→ general numpy fallback)."""
    base = page_table[:, :1]
    expect = base + np.arange(PAGES_PER_SEQ, dtype=page_table.dtype)[None, :]
    if np.array_equal(page_table, expect):
        return page_table[:, 0].astype(np.int64)
    return None


def _numpy_fallback(k, v, k_cache, v_cache, page_table, cache_seqlens):
    abs_pos = cache_seqlens[:, None].astype(np.int64) + np.arange(T)[None, :]
    page_idx = np.take_along_axis(
        page_table.astype(np.int64), abs_pos // PAGE_SIZE, axis=1
    )
    slot = abs_pos % PAGE_SIZE
    new_k = np.array(k_cache, copy=True)
    new_v = np.array(v_cache, copy=True)
    new_k[page_idx, slot] = k
    new_v[page_idx, slot] = v
    return new_k, new_v, (cache_seqlens + np.int32(T)).astype(np.int32)


def _make_in_maps(k, v, cache_seqlens):
    seql = cache_seqlens.astype(np.int64)
    kf = k.reshape(B, T * ROW)
    vf = v.reshape(B, T * ROW)
    in_maps = []
    for c in range(N_CORES):
        gb = slice(c * SPC, (c + 1) * SPC)
        dst = (np.arange(SPC, dtype=np.int64) * REGION + seql[gb]).astype(np.int32)
        kv = np.stack([kf[gb].reshape(SPC * T, ROW), vf[gb].reshape(SPC * T, ROW)])
        in_maps.append({"kv_src": kv, "dst_row": dst.reshape(1, SPC)})
    return in_maps


def kernel(k, v, k_cache, v_cache, page_table, cache_seqlens):
    k = np.ascontiguousarray(k, dtype=np.float32)
    v = np.ascontiguousarray(v, dtype=np.float32)
    page_table = np.asarray(page_table)
    cache_seqlens = np.asarray(cache_seqlens)

    base_pages = _seq_page_base(page_table)
    if base_pages is None:
        return _numpy_fallback(k, v, k_cache, v_cache, page_table, cache_seqlens)

    nc = _get_program()
    seql = cache_seqlens.astype(np.int64)
    in_maps = _make_in_maps(k, v, cache_seqlens)
    res = run_bass_kernel_spmd(nc, in_maps, core_ids=list(range(N_CORES)))

    new_k = np.array(k_cache, copy=True).reshape(NUM_PAGES * PAGE_SIZE, ROW)
    new_v = np.array(v_cache, copy=True).reshape(NUM_PAGES * PAGE_SIZE, ROW)
    for c in range(N_CORES):
        kv_out = res.results[c]["kv_out"]
        for bl in range(SPC):
            g = c * SPC + bl
            s = int(seql[g])
            dst0 = int(base_pages[g]) * PAGE_SIZE + s
            src0 = bl * REGION + s
            new_k[dst0 : dst0 + T] = kv_out[0, src0 : src0 + T]
            new_v[dst0 : dst0 + T] = kv_out[1, src0 : src0 + T]

    new_k = new_k.reshape(NUM_PAGES, PAGE_SIZE, H, D)
    new_v = new_v.reshape(NUM_PAGES, PAGE_SIZE, H, D)
    new_seqlens = (cache_seqlens + np.int32(T)).astype(np.int32)
    return new_k, new_v, new_seqlens


# revision 9
# speedup vs baseline: 1077.9159x; 3.8684x over previous
"""KV cache paged scatter update (nn_KVCacheManager) on 8 TRN2 NeuronCores.

Reference semantics (B=32, T=512, H=8, D=128, PAGE_SIZE=256):
  for each (b, i): abs = cache_seqlens[b] + i
    page = page_table[b, abs // 256]; slot = abs % 256
    k_cache[page, slot] = k[b, i]; v_cache[page, slot] = v[b, i]
  new_seqlens = cache_seqlens + T

Sharding: data parallel over batch — core c owns sequences 4c..4c+3 and
their pages, scattering locally with zero communication.

Device kernel (one SPMD program, offsets are runtime DATA, not constants):
  Each sequence writes T=512 consecutive tokens starting at abs offset
  cache_seqlens[b]. When a sequence's pages are consecutive in the page
  table (they are: page_table is arange), its destination is one
  contiguous 512-row range inside its private 2048-row region. So the
  scatter is one dynamic-offset 4 MiB copy per sequence (k and v stacked
  on the AP's outer dim), with the row offset loaded from an int32 input
  tensor into an SP register at runtime and hardware-bounds-checked.
Host does only what a production paged-KV host does: computes the int32
slot mapping (a few hundred ints) and assembles the final numpy output.
"""

import numpy as np

import concourse.bass as bass
import concourse.mybir as mybir
from concourse.bass_utils import run_bass_kernel_spmd

B, T, H, D = 32, 512, 8, 128
PAGE_SIZE = 256
PAGES_PER_SEQ = 8
NUM_PAGES = B * PAGES_PER_SEQ  # 256
N_CORES = 8
SPC = B // N_CORES             # 4 sequences per core
REGION = PAGES_PER_SEQ * PAGE_SIZE  # 2048 rows (slots) per sequence
ROW = H * D                    # 1024 f32 per slot (4 KiB)

_PROGRAM = None  # compiled Bass program cache (module-level, one build)


def _build_program():
    """One SPMD program for all 8 cores. Per-core behavior differs only in
    input data: the kv shard and the dst_row offsets.

    kv_src[0] = k tokens, kv_src[1] = v tokens (SPC*T rows of 4 KiB each).
    kv_out[0] = k cache regions, kv_out[1] = v cache regions
    (SPC sequences x 2048 rows). ExternalOutput buffers are zero-filled by
    the runtime; only the written rows are consumed by the host. One DMA
    per sequence moves both k and v (outer AP dim) to the dynamic row
    offset. Few DMA instructions matters: the TileContext tail drain
    encodes one sync-wait per DMA sem lane and overflows past ~8.
    """
    nc = bass.Bass()
    f32 = mybir.dt.float32
    i32 = mybir.dt.int32

    kv_src = nc.dram_tensor("kv_src", (2, SPC * T, ROW), f32, kind="ExternalInput")
    # dst_row[0, b] = b*REGION + cache_seqlens[global b]
    dst_row = nc.dram_tensor("dst_row", (1, SPC), i32, kind="ExternalInput")
    kv_out = nc.dram_tensor(
        "kv_out", (2, SPC * REGION, ROW), f32, kind="ExternalOutput"
    )

    # Raw Bass (no TileContext): the Tile tail drain encodes one sync-wait
    # per outstanding sem and overflows the TPB_CTRL wait-slot limit with
    # more than one dynamic DMA. Here everything runs on the gpsimd (Pool)
    # sequencer in program order with one shared DMA semaphore.
    #
    # SWDGE (gpsimd) is the only DGE whose DRAM->DRAM descriptors spray
    # across many SDMA engines; HWDGE pins them to 2 engines (~54 GB/s).
    # Splitting each sequence's 4 MiB into two 2 MiB DMAs engages all 16
    # SDMA engines (~280 GB/s payload). More/smaller DMAs exhaust the 48
    # Pool registers (each dynamic AP consumes scratch registers).
    HT = T // 2
    with (
        nc.sbuf_tensor([1, SPC], i32) as off_tile,
        nc.semaphore() as dma_sem,
        nc.gpsimd.register("o0") as r0,
        nc.gpsimd.register("o1") as r1,
        nc.gpsimd.register("o2") as r2,
        nc.gpsimd.register("o3") as r3,
        nc.gpsimd.register("h0") as h0,
        nc.gpsimd.register("h1") as h1,
        nc.gpsimd.register("h2") as h2,
        nc.gpsimd.register("h3") as h3,
        nc.Block() as block,
    ):
        regs = [r0, r1, r2, r3]
        hregs = [h0, h1, h2, h3]

        @block.gpsimd
        def _(gp):
            gp.dma_start(off_tile[:, :], dst_row[:, :]).then_inc(dma_sem, 16)
            gp.wait_ge(dma_sem, 16)
            for b in range(SPC):
                gp.reg_load(regs[b], off_tile[0:1, b : b + 1])
                gp.reg_add(hregs[b], gp.snap(regs[b]), HT)
                rv = gp.snap(regs[b])
                rvh = gp.snap(hregs[b])
                # Dynamic DRAM dst offsets: dma_start auto-enables the
                # hardware bounds check (bounds_check="err").
                gp.dma_start(
                    kv_out[:, bass.ds(rv, HT), :],
                    kv_src[:, b * T : b * T + HT, :],
                ).then_inc(dma_sem, 16)
                gp.dma_start(
                    kv_out[:, bass.ds(rvh, HT), :],
                    kv_src[:, b * T + HT : (b + 1) * T, :],
                ).then_inc(dma_sem, 16)
            gp.wait_ge(dma_sem, 16 * (2 * SPC + 1))

    return nc


def _get_program():
    global _PROGRAM
    if _PROGRAM is None:
        _PROGRAM = _build_program()
    return _PROGRAM


def _seq_page_base(page_table):
    """If every sequence's pages are consecutive (page_table[b, j] ==
    page_table[b, 0] + j), return the (B,) int array of base pages, else
    None (→ general numpy fallback)."""
    base = page_table[:, :1]
    expect = base + np.arange(PAGES_PER_SEQ, dtype=page_table.dtype)[None, :]
    if np.array_equal(page_table, expect):
        return page_table[:, 0].astype(np.int64)
    return None


def _numpy_fallback(k, v, k_cache, v_cache, page_table, cache_seqlens):
    abs_pos = cache_seqlens[:, None].astype(np.int64) + np.arange(T)[None, :]
    page_idx = np.take_along_axis(
        page_table.astype(np.int64), abs_pos // PAGE_SIZE, axis=1
    )
    slot = abs_pos % PAGE_SIZE
    new_k = np.array(k_cache, copy=True)
    new_v = np.array(v_cache, copy=True)
    new_k[page_idx, slot] = k
    new_v[page_idx, slot] = v
    return new_k, new_v, (cache_seqlens + np.int32(T)).astype(np.int32)


def _make_in_maps(k, v, cache_seqlens):
    seql = cache_seqlens.astype(np.int64)
    kf = k.reshape(B, T * ROW)
    vf = v.reshape(B, T * ROW)
    in_maps = []
    for c in range(N_CORES):
        gb = slice(c * SPC, (c + 1) * SPC)
        dst = (np.arange(SPC, dtype=np.int64) * REGION + seql[gb]).astype(np.int32)
        kv = np.stack([kf[gb].reshape(SPC * T, ROW), vf[gb].reshape(SPC * T, ROW)])
        in_maps.append({"kv_src": kv, "dst_row": dst.reshape(1, SPC)})
    return in_maps


def kernel(k, v, k_cache, v_cache, page_table, cache_seqlens):
    k = np.ascontiguousarray(k, dtype=np.float32)
    v = np.ascontiguousarray(v, dtype=np.float32)
    page_table = np.asarray(page_table)
    cache_seqlens = np.asarray(cache_seqlens)

    base_pages = _seq_page_base(page_table)
    if base_pages is None:
        return _numpy_fallback(k, v, k_cache, v_cache, page_table, cache_seqlens)

    nc = _get_program()
    seql = cache_seqlens.astype(np.int64)
    in_maps = _make_in_maps(k, v, cache_seqlens)
    res = run_bass_kernel_spmd(nc, in_maps, core_ids=list(range(N_CORES)))

    new_k = np.array(k_cache, copy=True).reshape(NUM_PAGES * PAGE_SIZE, ROW)
    new_v = np.array(v_cache, copy=True).reshape(NUM_PAGES * PAGE_SIZE, ROW)
    for c in range(N_CORES):
        kv_out = res.results[c]["kv_out"]
        for bl in range(SPC):
            g = c * SPC + bl
            s = int(seql[g])
            dst0 = int(base_pages[g]) * PAGE_SIZE + s
            src0 = bl * REGION + s
            new_k[dst0 : dst0 + T] = kv_out[0, src0 : src0 + T]
            new_v[dst0 : dst0 + T] = kv_out[1, src0 : src0 + T]

    new_k = new_k.reshape(NUM_PAGES, PAGE_SIZE, H, D)
    new_v = new_v.reshape(NUM_PAGES, PAGE_SIZE, H, D)
    new_seqlens = (cache_seqlens + np.int32(T)).astype(np.int32)
    return new_k, new_v, new_seqlens


# revision 10
# speedup vs baseline: 1158.0395x; 1.0743x over previous
"""KV cache paged scatter update (nn_KVCacheManager) on 8 TRN2 NeuronCores.

Reference semantics (B=32, T=512, H=8, D=128, PAGE_SIZE=256):
  for each (b, i): abs = cache_seqlens[b] + i
    page = page_table[b, abs // 256]; slot = abs % 256
    k_cache[page, slot] = k[b, i]; v_cache[page, slot] = v[b, i]
  new_seqlens = cache_seqlens + T

Sharding: data parallel over batch — core c owns sequences 4c..4c+3 and
their pages, scattering locally with zero communication.

Device kernel (one SPMD program, offsets are runtime DATA, not constants):
  Each sequence writes T=512 consecutive tokens starting at abs offset
  cache_seqlens[b]. When a sequence's pages are consecutive in the page
  table (they are: page_table is arange), its destination is one
  contiguous 512-row range inside its private 2048-row region. So the
  scatter is one dynamic-offset 4 MiB copy per sequence (k and v stacked
  on the AP's outer dim), with the row offset loaded from an int32 input
  tensor into an SP register at runtime and hardware-bounds-checked.
Host does only what a production paged-KV host does: computes the int32
slot mapping (a few hundred ints) and assembles the final numpy output.
"""

import numpy as np

import concourse.bass as bass
import concourse.mybir as mybir
from concourse.bass_utils import run_bass_kernel_spmd

B, T, H, D = 32, 512, 8, 128
PAGE_SIZE = 256
PAGES_PER_SEQ = 8
NUM_PAGES = B * PAGES_PER_SEQ  # 256
N_CORES = 8
SPC = B // N_CORES             # 4 sequences per core
REGION = PAGES_PER_SEQ * PAGE_SIZE  # 2048 rows (slots) per sequence
ROW = H * D                    # 1024 f32 per slot (4 KiB)

_PROGRAM = None  # compiled Bass program cache (module-level, one build)


def _build_program():
    """One SPMD program for all 8 cores. Per-core behavior differs only in
    input data: the kv shard and the dst_row offsets.

    kv_src[0] = k tokens, kv_src[1] = v tokens (SPC*T rows of 4 KiB each).
    kv_out[0] = k cache regions, kv_out[1] = v cache regions
    (SPC sequences x 2048 rows). ExternalOutput buffers are zero-filled by
    the runtime; only the written rows are consumed by the host. One DMA
    per sequence moves both k and v (outer AP dim) to the dynamic row
    offset. Few DMA instructions matters: the TileContext tail drain
    encodes one sync-wait per DMA sem lane and overflows past ~8.
    """
    nc = bass.Bass()
    f32 = mybir.dt.float32
    i32 = mybir.dt.int32

    kv_src = nc.dram_tensor("kv_src", (2, SPC * T, ROW), f32, kind="ExternalInput")
    # dst_row[0, b] = b*REGION + cache_seqlens[global b]
    dst_row = nc.dram_tensor("dst_row", (1, SPC), i32, kind="ExternalInput")
    kv_out = nc.dram_tensor(
        "kv_out", (2, SPC * REGION, ROW), f32, kind="ExternalOutput"
    )

    # Raw Bass (no TileContext): the Tile tail drain encodes one sync-wait
    # per outstanding sem and overflows the TPB_CTRL wait-slot limit with
    # more than one dynamic DMA. Here everything runs on the gpsimd (Pool)
    # sequencer in program order with one shared DMA semaphore.
    #
    # SWDGE (gpsimd) is the only DGE whose DRAM->DRAM descriptors spray
    # across all 16 SDMA engines; HWDGE pins them to 2 engines (~54 GB/s).
    # 16 x 1 MiB DMAs (seq-half x cache) reuse 8 offset register values
    # (donated snaps); the SWDGE balancer equalizes bytes per engine.
    # Measured ~80 us on 8 cores (~280 GB/s payload per core, bounded by
    # aggregate SDMA engine throughput for DRAM->DRAM round trips).
    HT = T // 2
    with (
        nc.sbuf_tensor([1, SPC], i32) as off_tile,
        nc.semaphore() as dma_sem,
        nc.Block() as block,
    ):

        @block.gpsimd
        def _(gp):
            gp.dma_start(off_tile[:, :], dst_row[:, :]).then_inc(dma_sem, 16)
            gp.wait_ge(dma_sem, 16)
            offs = []  # [b] -> (row offset, row offset + HT) ScalarValues
            for b in range(SPC):
                r = gp.alloc_register(f"o{b}")
                hreg = gp.alloc_register(f"h{b}")
                gp.reg_load(r, off_tile[0:1, b : b + 1])
                sv = gp.snap(r, donate=True)
                gp.reg_add(hreg, sv, HT)
                svh = gp.snap(hreg, donate=True)
                offs.append((sv, svh))
            n = 1
            for b in range(SPC):
                for q, off in enumerate(offs[b]):
                    for h in range(2):
                        # Dynamic DRAM dst offsets: dma_start auto-enables
                        # the hardware bounds check (bounds_check="err").
                        gp.dma_start(
                            kv_out[h, bass.ds(off, HT), :],
                            kv_src[h, b * T + q * HT : b * T + (q + 1) * HT, :],
                        ).then_inc(dma_sem, 16)
                        n += 1
            gp.wait_ge(dma_sem, 16 * n)

    return nc


def _get_program():
    global _PROGRAM
    if _PROGRAM is None:
        _PROGRAM = _build_program()
    return _PROGRAM


def _seq_page_base(page_table):
    """If every sequence's pages are consecutive (page_table[b, j] ==
    page_table[b, 0] + j), return the (B,) int array of base pages, else
    None (→ general numpy fallback)."""
    base = page_table[:, :1]
    expect = base + np.arange(PAGES_PER_SEQ, dtype=page_table.dtype)[None, :]
    if np.array_equal(page_table, expect):
        return page_table[:, 0].astype(np.int64)
    return None


def _numpy_fallback(k, v, k_cache, v_cache, page_table, cache_seqlens):
    abs_pos = cache_seqlens[:, None].astype(np.int64) + np.arange(T)[None, :]
    page_idx = np.take_along_axis(
        page_table.astype(np.int64), abs_pos // PAGE_SIZE, axis=1
    )
    slot = abs_pos % PAGE_SIZE
    new_k = np.array(k_cache, copy=True)
    new_v = np.array(v_cache, copy=True)
    new_k[page_idx, slot] = k
    new_v[page_idx, slot] = v
    return new_k, new_v, (cache_seqlens + np.int32(T)).astype(np.int32)


def _make_in_maps(k, v, cache_seqlens):
    seql = cache_seqlens.astype(np.int64)
    kf = k.reshape(B, T * ROW)
    vf = v.reshape(B, T * ROW)
    in_maps = []
    for c in range(N_CORES):
        gb = slice(c * SPC, (c + 1) * SPC)
        dst = (np.arange(SPC, dtype=np.int64) * REGION + seql[gb]).astype(np.int32)
        kv = np.stack([kf[gb].reshape(SPC * T, ROW), vf[gb].reshape(SPC * T, ROW)])
        in_maps.append({"kv_src": kv, "dst_row": dst.reshape(1, SPC)})
    return in_maps


def kernel(k, v, k_cache, v_cache, page_table, cache_seqlens):
    k = np.ascontiguousarray(k, dtype=np.float32)
    v = np.ascontiguousarray(v, dtype=np.float32)
    page_table = np.asarray(page_table)
    cache_seqlens = np.asarray(cache_seqlens)

    base_pages = _seq_page_base(page_table)
    if base_pages is None:
        return _numpy_fallback(k, v, k_cache, v_cache, page_table, cache_seqlens)

    nc = _get_program()
    seql = cache_seqlens.astype(np.int64)
    in_maps = _make_in_maps(k, v, cache_seqlens)
    res = run_bass_kernel_spmd(nc, in_maps, core_ids=list(range(N_CORES)))

    new_k = np.array(k_cache, copy=True).reshape(NUM_PAGES * PAGE_SIZE, ROW)
    new_v = np.array(v_cache, copy=True).reshape(NUM_PAGES * PAGE_SIZE, ROW)
    for c in range(N_CORES):
        kv_out = res.results[c]["kv_out"]
        for bl in range(SPC):
            g = c * SPC + bl
            s = int(seql[g])
            dst0 = int(base_pages[g]) * PAGE_SIZE + s
            src0 = bl * REGION + s
            new_k[dst0 : dst0 + T] = kv_out[0, src0 : src0 + T]
            new_v[dst0 : dst0 + T] = kv_out[1, src0 : src0 + T]

    new_k = new_k.reshape(NUM_PAGES, PAGE_SIZE, H, D)
    new_v = new_v.reshape(NUM_PAGES, PAGE_SIZE, H, D)
    new_seqlens = (cache_seqlens + np.int32(T)).astype(np.int32)
    return new_k, new_v, new_seqlens


# revision 11
# speedup vs baseline: 1358.0484x; 1.1727x over previous
"""KV cache paged scatter update (nn_KVCacheManager) on 8 TRN2 NeuronCores.

Reference semantics (B=32, T=512, H=8, D=128, PAGE_SIZE=256):
  for each (b, i): abs = cache_seqlens[b] + i
    page = page_table[b, abs // 256]; slot = abs % 256
    k_cache[page, slot] = k[b, i]; v_cache[page, slot] = v[b, i]
  new_seqlens = cache_seqlens + T

Sharding: data parallel over batch — core c owns sequences 4c..4c+3 and
their pages, scattering locally with zero communication.

Device kernel (one SPMD program, offsets are runtime DATA, not constants):
  Each sequence writes T=512 consecutive tokens starting at abs offset
  cache_seqlens[b]. When a sequence's pages are consecutive in the page
  table (they are: page_table is arange), its destination is one
  contiguous 512-row range inside its private 2048-row region. So the
  scatter is one dynamic-offset 4 MiB copy per sequence (k and v stacked
  on the AP's outer dim), with the row offset loaded from an int32 input
  tensor into an SP register at runtime and hardware-bounds-checked.
Host does only what a production paged-KV host does: computes the int32
slot mapping (a few hundred ints) and assembles the final numpy output.
"""

import numpy as np

import concourse.bass as bass
import concourse.mybir as mybir
from concourse.bass_utils import run_bass_kernel_spmd

B, T, H, D = 32, 512, 8, 128
PAGE_SIZE = 256
PAGES_PER_SEQ = 8
NUM_PAGES = B * PAGES_PER_SEQ  # 256
N_CORES = 8
SPC = B // N_CORES             # 4 sequences per core
REGION = PAGES_PER_SEQ * PAGE_SIZE  # 2048 rows (slots) per sequence
ROW = H * D                    # 1024 f32 per slot (4 KiB)

_PROGRAM = None  # compiled Bass program cache (module-level, one build)


def _build_program():
    """One SPMD program for all 8 cores. Per-core behavior differs only in
    input data: the kv shard and the dst_row offsets.

    kv_src[0] = k tokens, kv_src[1] = v tokens (SPC*T rows of 4 KiB each).
    kv_out[0] = k cache regions, kv_out[1] = v cache regions
    (SPC sequences x 2048 rows). ExternalOutput buffers are zero-filled by
    the runtime; only the written rows are consumed by the host. One DMA
    per sequence moves both k and v (outer AP dim) to the dynamic row
    offset. Few DMA instructions matters: the TileContext tail drain
    encodes one sync-wait per DMA sem lane and overflows past ~8.
    """
    nc = bass.Bass()
    f32 = mybir.dt.float32
    i32 = mybir.dt.int32

    kv_src = nc.dram_tensor("kv_src", (2, SPC * T, ROW), f32, kind="ExternalInput")
    # dst_row[0, b] = b*REGION + cache_seqlens[global b]
    dst_row = nc.dram_tensor("dst_row", (1, SPC), i32, kind="ExternalInput")
    kv_out = nc.dram_tensor(
        "kv_out", (2, SPC * REGION, ROW), f32, kind="ExternalOutput"
    )

    # Raw Bass (no TileContext): the Tile tail drain encodes one sync-wait
    # per outstanding sem and overflows the TPB_CTRL wait-slot limit with
    # more than one dynamic DMA. Here everything runs on the gpsimd (Pool)
    # sequencer in program order with one shared DMA semaphore.
    #
    # SWDGE (gpsimd) is the only DGE whose DRAM->DRAM descriptors spray
    # across all 16 SDMA engines; HWDGE pins them to 2 engines (~54 GB/s).
    # 16 x 1 MiB DMAs (seq-half x cache) reuse 8 offset register values
    # (donated snaps); the SWDGE balancer equalizes bytes per engine.
    # Measured ~80 us on 8 cores (~280 GB/s payload per core, bounded by
    # aggregate SDMA engine throughput for DRAM->DRAM round trips).
    HT = T // 2
    with (
        nc.semaphore() as dma_sem,
        nc.Block() as block,
    ):

        @block.gpsimd
        def _(gp):
            # One batched register load straight from DRAM — ~3 us faster
            # than DMA-to-SBUF + sem wait + per-value loads. Must be one
            # call: per-value DRAM loads each burn 64-bit address register
            # pairs and overflow the 48-register Pool file.
            regs = [gp.alloc_register(f"o{b}") for b in range(SPC)]
            gp.reg_load(regs, dst_row[0:1, 0:SPC])
            offs = []  # [b] -> (row offset, row offset + HT) ScalarValues
            for b in range(SPC):
                hreg = gp.alloc_register(f"h{b}")
                sv = gp.snap(regs[b], donate=True)
                gp.reg_add(hreg, sv, HT)
                svh = gp.snap(hreg, donate=True)
                offs.append((sv, svh))
            n = 0
            for b in range(SPC):
                for q, off in enumerate(offs[b]):
                    for h in range(2):
                        # Dynamic DRAM dst offsets: dma_start auto-enables
                        # the hardware bounds check (bounds_check="err").
                        gp.dma_start(
                            kv_out[h, bass.ds(off, HT), :],
                            kv_src[h, b * T + q * HT : b * T + (q + 1) * HT, :],
                        ).then_inc(dma_sem, 16)
                        n += 1
            gp.wait_ge(dma_sem, 16 * n)

    return nc


def _get_program():
    global _PROGRAM
    if _PROGRAM is None:
        _PROGRAM = _build_program()
    return _PROGRAM


def _seq_page_base(page_table):
    """If every sequence's pages are consecutive (page_table[b, j] ==
    page_table[b, 0] + j), return the (B,) int array of base pages, else
    None (→ general numpy fallback)."""
    base = page_table[:, :1]
    expect = base + np.arange(PAGES_PER_SEQ, dtype=page_table.dtype)[None, :]
    if np.array_equal(page_table, expect):
        return page_table[:, 0].astype(np.int64)
    return None


def _numpy_fallback(k, v, k_cache, v_cache, page_table, cache_seqlens):
    abs_pos = cache_seqlens[:, None].astype(np.int64) + np.arange(T)[None, :]
    page_idx = np.take_along_axis(
        page_table.astype(np.int64), abs_pos // PAGE_SIZE, axis=1
    )
    slot = abs_pos % PAGE_SIZE
    new_k = np.array(k_cache, copy=True)
    new_v = np.array(v_cache, copy=True)
    new_k[page_idx, slot] = k
    new_v[page_idx, slot] = v
    return new_k, new_v, (cache_seqlens + np.int32(T)).astype(np.int32)


def _make_in_maps(k, v, cache_seqlens):
    seql = cache_seqlens.astype(np.int64)
    kf = k.reshape(B, T * ROW)
    vf = v.reshape(B, T * ROW)
    in_maps = []
    for c in range(N_CORES):
        gb = slice(c * SPC, (c + 1) * SPC)
        dst = (np.arange(SPC, dtype=np.int64) * REGION + seql[gb]).astype(np.int32)
        kv = np.stack([kf[gb].reshape(SPC * T, ROW), vf[gb].reshape(SPC * T, ROW)])
        in_maps.append({"kv_src": kv, "dst_row": dst.reshape(1, SPC)})
    return in_maps


def kernel(k, v, k_cache, v_cache, page_table, cache_seqlens):
    k = np.ascontiguousarray(k, dtype=np.float32)
    v = np.ascontiguousarray(v, dtype=np.float32)
    page_table = np.asarray(page_table)
    cache_seqlens = np.asarray(cache_seqlens)

    base_pages = _seq_page_base(page_table)
    if base_pages is None:
        return _numpy_fallback(k, v, k_cache, v_cache, page_table, cache_seqlens)

    nc = _get_program()
    seql = cache_seqlens.astype(np.int64)
    in_maps = _make_in_maps(k, v, cache_seqlens)
    res = run_bass_kernel_spmd(nc, in_maps, core_ids=list(range(N_CORES)))

    new_k = np.array(k_cache, copy=True).reshape(NUM_PAGES * PAGE_SIZE, ROW)
    new_v = np.array(v_cache, copy=True).reshape(NUM_PAGES * PAGE_SIZE, ROW)
    for c in range(N_CORES):
        kv_out = res.results[c]["kv_out"]
        for bl in range(SPC):
            g = c * SPC + bl
            s = int(seql[g])
            dst0 = int(base_pages[g]) * PAGE_SIZE + s
            src0 = bl * REGION + s
            new_k[dst0 : dst0 + T] = kv_out[0, src0 : src0 + T]
            new_v[dst0 : dst0 + T] = kv_out[1, src0 : src0 + T]

    new_k = new_k.reshape(NUM_PAGES, PAGE_SIZE, H, D)
    new_v = new_v.reshape(NUM_PAGES, PAGE_SIZE, H, D)
    new_seqlens = (cache_seqlens + np.int32(T)).astype(np.int32)
    return new_k, new_v, new_seqlens
